# revision 1
# baseline (speedup 1.0000x reference)
"""QKV projection (qkv = hidden_states @ qkv_proj.T -> q, k, v heads) on
8 TRN2 NeuronCores.

Sharding: data-parallel over tokens (16384 rows / 8 cores); qkv_proj
replicated. Per-core GEMM [2048, 4096] @ [4096, 12288] runs as a
mixed-precision K-hybrid: k-tiles 0..23 (3072 k's) in fp16 at 1 cy/row,
k-tiles 24..31 (1024 k's) in fp8e4 with perf_mode=DoubleRow at ~0.5 cy/row
(measured 222 ns per K=256,N=512 matmul vs 216 ns for fp16 K=128), all
accumulating into the same fp32 PSUM group. Measured end-to-end rel err
1.88e-2 (fp8 quantization noise scaled by sqrt(1024/4096)), under the 2e-2
gate. W is pre-scaled by 64 (exact power of 2) so the fp8 weight values
sit in e4m3's normal range; the host divides the output by 64.

DRAM layouts are pre-tiled on host so every DMA is contiguous:
  x16  [128, 24, 2048]  : x16[p, ko, m] = hidden[m_global, ko*128+p]  fp16
  x8   [128,  8, 2048]  : same for ko 24..31                          fp8e4
  w16  [128, 24, 12288] : w16[p, ko, n] = 64*qkv_proj[n, ko*128+p]    fp16
  w8   [128,  8, 12288] : same for ko 24..31                          fp8e4
  outt [128, 96, 2048]  : outt[p, nb, m] = 64*qkv[m_global, nb*128+p] fp32

Warmup DMA pacing: only a small first x/W piece is in flight at t=0; later
input DMAs are released by PE progress via explicit dep edges so the t=0
round-robin burst never starves the operand the next matmul needs. Output
DMAs ride the ACT HWDGE ring to keep them off the input ring's FIFO."""

import sys
import types

import numpy as np
import ml_dtypes

try:
    import antenv.axon_hooks  # noqa: F401
except ImportError:
    import antenv

    _m = types.ModuleType("antenv.axon_hooks")
    _m._hook = None
    _m.set_axon_ntff_profile_hook = lambda h: setattr(_m, "_hook", h)
    _m.get_axon_ntff_profile_hook = lambda: _m._hook
    sys.modules["antenv.axon_hooks"] = _m
    antenv.axon_hooks = _m

import concourse.bacc as bacc
import concourse.mybir as mybir
import concourse.tile as tile
from concourse.tile import add_dep_helper
from concourse._compat import get_trn_type
from concourse.bass_utils import run_bass_kernel_spmd

P = 128
EMBED = 4096
KO = EMBED // P          # 32
KO16 = 24                # k-tiles in fp16
KO8 = KO - KO16          # 8 k-tiles in fp8 (4 DoubleRow pairs)
NP8 = KO8 // 2           # 4 pairs
NQKV = 3 * EMBED
TOKENS = 16384
N_CORES = 8
M_CORE = TOKENS // N_CORES  # 2048
NB = NQKV // P              # 96
MS = 512
XCH = 4                     # k-subtiles per fp16 x chunk -> 6 chunks of 2MB
WSCALE = 64.0

f32 = mybir.dt.float32
f16 = mybir.dt.float16
f8 = mybir.dt.float8e4
F16 = np.float16
F8 = ml_dtypes.float8_e4m3
DR = mybir.MatmulPerfMode.DoubleRow

_CACHE = {}
LAST_RESULTS = None


def _build():
    nc = bacc.Bacc(get_trn_type() or "TRN2", target_bir_lowering=False, debug=False)
    x16_d = nc.dram_tensor("x16", (P, KO16, M_CORE), f16, kind="ExternalInput")
    x8_d = nc.dram_tensor("x8", (P, KO8, M_CORE), f8, kind="ExternalInput")
    w16_d = nc.dram_tensor("w16", (P, KO16, NQKV), f16, kind="ExternalInput")
    w8_d = nc.dram_tensor("w8", (P, KO8, NQKV), f8, kind="ExternalInput")
    out_d = nc.dram_tensor("outt", (P, NB, M_CORE), f32, kind="ExternalOutput")

    NCH = KO16 // XCH  # 6
    with tile.TileContext(nc) as tc:
        with tc.tile_pool(name="xpool", bufs=1) as xpool, \
             tc.tile_pool(name="wpool", bufs=8) as wpool, \
             tc.tile_pool(name="w8pool", bufs=8) as w8pool, \
             tc.tile_pool(name="pspool", bufs=8, space="PSUM") as pspool, \
             tc.tile_pool(name="opool", bufs=6) as opool:
            x_ch = []
            x_dmas = []
            for c in range(NCH):
                xc = xpool.tile([P, XCH, M_CORE], f16, tag=f"x{c}",
                                name=f"x_ch{c}")
                if c == 0:
                    # split chunk 0 so the first matmul's operand region
                    # lands in ~a quarter of the time
                    dma = nc.sync.dma_start(xc[:, 0:1, :], x16_d[:, 0:1, :])
                    nc.sync.dma_start(xc[:, 1:XCH, :], x16_d[:, 1:XCH, :])
                else:
                    dma = nc.sync.dma_start(
                        xc[:], x16_d[:, c * XCH:(c + 1) * XCH, :]
                    )
                x_ch.append(xc)
                x_dmas.append(dma)
            x8t = xpool.tile([P, KO8, M_CORE], f8, tag="x8", name="x8t")
            x8_dma = nc.sync.dma_start(x8t[:], x8_d[:])

            w_dmas = []
            w8_dmas = []
            nb0_ko_mm = {}     # first (ms=0) matmul consuming fp16 ko (warmup)
            nb_first_mm = {}   # first matmul of each nb
            nms = M_CORE // MS  # 4
            group = {}         # per-nb state for the interleaved warmup pair

            def emit_w_dmas(nb):
                wt = wpool.tile([P, KO16, P], f16, tag="w", name="w_t")
                if nb == 0:
                    # split W0 so the first matmuls' k-rows land first
                    w_dmas.append(
                        nc.sync.dma_start(wt[:, :4], w16_d[:, :4, :P])
                    )
                    nc.sync.dma_start(wt[:, 4:], w16_d[:, 4:, :P])
                else:
                    w_dmas.append(
                        nc.sync.dma_start(wt[:], w16_d[:, :, nb * P:(nb + 1) * P])
                    )
                wt8 = w8pool.tile([P, KO8, P], f8, tag="w8", name="w8_t")
                w8_dmas.append(
                    nc.sync.dma_start(wt8[:], w8_d[:, :, nb * P:(nb + 1) * P])
                )
                pss = [
                    pspool.tile([P, MS], f32, tag="ps", name="ps")
                    for _ in range(nms)
                ]
                group[nb] = (wt, wt8, pss)

            def emit_mm(nb, kind, idx, ms):
                wt, wt8, pss = group[nb]
                if kind == "f":
                    xc = x_ch[idx // XCH]
                    kk = idx % XCH
                    mm = nc.tensor.matmul(
                        pss[ms][:],
                        wt[:, idx],
                        xc[:, kk, ms * MS:(ms + 1) * MS],
                        start=(idx == 0),
                        stop=False,
                    )
                    if nb == 0 and ms == 0:
                        nb0_ko_mm.setdefault(idx, mm)
                    if idx == 0 and ms == 0:
                        nb_first_mm.setdefault(nb, mm)
                else:
                    nc.tensor.matmul(
                        pss[ms][:],
                        wt8[:, 2 * idx:2 * idx + 2, :],
                        x8t[:, 2 * idx:2 * idx + 2,
                            ms * MS:(ms + 1) * MS],
                        start=False,
                        stop=(idx == NP8 - 1),
                        perf_mode=DR,
                    )

            def emit_drain(nb):
                _, _, pss = group[nb]
                for ms in range(nms):
                    o_sb = opool.tile([P, MS], f32, tag="o", name="o_sb")
                    nc.vector.tensor_copy(o_sb[:], pss[ms][:])
                    # outputs go out on the ACT HWDGE ring so they never
                    # head-of-line-block the W/x input stream on SP's ring
                    nc.scalar.dma_start(
                        out_d[:, nb, ms * MS:(ms + 1) * MS],
                        o_sb[:],
                    )

            # Warmup phase: nb 0 and 1 interleaved chunk-by-chunk so the PE
            # fills nb0's x-chunk DMA-wait gaps with nb1 work on chunks that
            # already landed. DMA call order (SP FIFO) is identical to the
            # plain loop: W0, w8_0, W1, w8_1 all precede later W tiles.
            emit_w_dmas(0)
            emit_w_dmas(1)
            for c in range(NCH):
                for nb in (0, 1):
                    for ko in range(c * XCH, (c + 1) * XCH):
                        for ms in range(nms):
                            emit_mm(nb, "f", ko, ms)
            for nb in (0, 1):
                for t in range(NP8):
                    for ms in range(nms):
                        emit_mm(nb, "q", t, ms)
            emit_drain(0)
            emit_drain(1)

            for nb in range(2, NB):
                emit_w_dmas(nb)
                # steps: fp16 kos 0..23 then fp8 pairs 0..3
                steps = [("f", ko) for ko in range(KO16)] + \
                        [("q", t) for t in range(NP8)]
                # last nb: ms-outer so the psum drains stagger and the final
                # drain tail is one group, not four
                if nb == NB - 1:
                    order = [(s, ms) for ms in range(nms) for s in steps]
                else:
                    order = [(s, ms) for s in steps for ms in range(nms)]
                for (kind, idx), ms in order:
                    emit_mm(nb, kind, idx, ms)
                emit_drain(nb)

            # Warmup pacing: only x0 + W0 are in flight at t=0 (so the first
            # matmul starts ~7us in); every later input DMA is released by PE
            # progress, staying ~2 chunks ahead of consumption.
            add_dep_helper(x_dmas[1].ins, nb0_ko_mm[0].ins, sync=True,
                           reason="x1 after first matmul")
            add_dep_helper(x_dmas[2].ins, nb0_ko_mm[0].ins, sync=True,
                           reason="x2 after first matmul")
            for c in range(3, NCH):
                add_dep_helper(x_dmas[c].ins, nb0_ko_mm[XCH * (c - 3)].ins,
                               sync=True, reason="pace x chunks off PE")
            add_dep_helper(x8_dma.ins, nb0_ko_mm[12].ins, sync=True,
                           reason="pace x8 chunk off PE")
            for j in range(0, 7):
                add_dep_helper(w_dmas[j + 1].ins, nb_first_mm[j].ins,
                               sync=True, reason="pace early W off PE")
                add_dep_helper(w8_dmas[j + 1].ins, nb_first_mm[j].ins,
                               sync=True, reason="pace early W8 off PE")
            add_dep_helper(w8_dmas[0].ins, nb0_ko_mm[0].ins, sync=True,
                           reason="W8_0 after first matmul")

    nc.compile()
    return nc


def kernel(hidden_states, qkv_proj, position_ids=None, **_unused):
    global LAST_RESULTS
    x = np.ascontiguousarray(hidden_states, dtype=np.float32).reshape(TOKENS, EMBED)
    w = np.ascontiguousarray(qkv_proj, dtype=np.float32)

    if "nc" not in _CACHE:
        _CACHE["nc"] = _build()
    nc = _CACHE["nc"]

    w_t = np.ascontiguousarray(
        (w * np.float32(WSCALE)).T.reshape(KO, P, NQKV).transpose(1, 0, 2)
    )
    w16 = w_t[:, :KO16].astype(F16)
    w8 = np.clip(w_t[:, KO16:], -240, 240).astype(F8)
    in_maps = []
    for i in range(N_CORES):
        xs = x[i * M_CORE:(i + 1) * M_CORE]
        x_t = np.ascontiguousarray(
            xs.T.reshape(KO, P, M_CORE).transpose(1, 0, 2)
        )
        x16 = x_t[:, :KO16].astype(F16)
        x8 = np.clip(x_t[:, KO16:], -240, 240).astype(F8)
        in_maps.append({"x16": x16, "x8": x8, "w16": w16, "w8": w8})

    res = run_bass_kernel_spmd(nc, in_maps, core_ids=list(range(N_CORES)))
    LAST_RESULTS = res

    inv = np.float32(1.0 / WSCALE)
    parts = [
        res.results[i]["outt"].transpose(2, 1, 0).reshape(M_CORE, NQKV) * inv
        for i in range(N_CORES)
    ]
    qkv = np.concatenate(parts, axis=0)
    query = np.ascontiguousarray(qkv[:, :EMBED]).reshape(TOKENS, 32, 128)
    key = np.ascontiguousarray(qkv[:, EMBED:2 * EMBED]).reshape(TOKENS, 32, 128)
    value = np.ascontiguousarray(qkv[:, 2 * EMBED:]).reshape(TOKENS, 32, 128)
    return (query, key, value)



# revision 3
# speedup vs baseline: 1.1697x; 1.1697x over previous
"""v4: 4 tile pools (shorter entry/exit barriers), gpsimd memsets.

QKV projection (qkv = hidden_states @ qkv_proj.T -> q, k, v heads) on
8 TRN2 NeuronCores.

Sharding: data-parallel over tokens (16384 rows / 8 cores); qkv_proj
replicated. Per-core GEMM [2048, 4096] @ [4096, 12288] runs as a
mixed-precision K-hybrid: k-tiles 0..23 (3072 k's) in fp16 at 1 cy/row,
k-tiles 24..31 (1024 k's) in fp8e4 with perf_mode=DoubleRow at ~0.5 cy/row,
all accumulating into the same fp32 PSUM group. 28 PE passes per output
tile is the precision-constrained floor: 27 passes would need >=1280 fp8
k's -> rel err ~2.1e-2 > the 2e-2 gate (e4m3's eps=2.65e-2/operand is
scale-invariant; e3m4+DoubleRow is rejected by the cayman ISA).

v3 ring plan (the v2 lesson: one ring cannot carry W+outputs at 81 GB/s):
  - sync (SP) ring:  x chunks 0,2,4 + x8 early, then the 384 output DMAs
    (which only start at ~45 us, after the x burst is done).
  - scalar (ACT) ring: W0/W1 heads first, x chunks 1,3,5 interleaved with
    the W0/W1 tails, then w8/W2..; steady W tiles stream one nb ahead,
    naturally paced by the wpool WAR backpressure (bufs=8). No explicit
    dep edges needed.
  - x on BOTH rings halves the warmup fill (x must fully land by ~L+48 us;
    stalls >3.4 us would re-throttle the HAM clock gate to half rate).
  - W pre-tiled nb-major on host ([P, NB, KO16, 128]) so each W-tile DMA
    is 128 rows of 6 KB instead of 3072 rows of 256 B.
  - PE prewarmed with junk N=64 matmuls on memset tiles so the HAM gate
    (4/8 -> 8/8 after ~3.4 us of activity) releases during the DMA wait.
  - W pre-scaled by 32 (not 64) so no e4m3 value clips at 240; host
    divides the output by 32.

DRAM layouts are pre-tiled on host so every DMA is contiguous:
  x16  [128, 24, 2048]     : x16[p, ko, m] = hidden[m_glob, ko*128+p] fp16
  x8   [128, 8, 2048]      : same for ko 24..31                       fp8e4
  w16  [128, 96, 24, 128]  : w16[p, nb, ko, n] = 32*W[nb*128+n, ko*128+p]
  w8   [128, 96, 8, 128]   : same for ko 24..31                       fp8e4
  outt [128, 96, 2048]     : outt[p, nb, m] = 32*qkv[m_glob, nb*128+p] f32
"""

import sys
import types

import numpy as np
import ml_dtypes

try:
    import antenv.axon_hooks  # noqa: F401
except ImportError:
    import antenv

    _m = types.ModuleType("antenv.axon_hooks")
    _m._hook = None
    _m.set_axon_ntff_profile_hook = lambda h: setattr(_m, "_hook", h)
    _m.get_axon_ntff_profile_hook = lambda: _m._hook
    sys.modules["antenv.axon_hooks"] = _m
    antenv.axon_hooks = _m

import concourse.bacc as bacc
import concourse.mybir as mybir
import concourse.tile as tile
from concourse._compat import get_trn_type
from concourse.bass_utils import run_bass_kernel_spmd

P = 128
EMBED = 4096
KO = EMBED // P          # 32
KO16 = 24                # k-tiles in fp16
KO8 = KO - KO16          # 8 k-tiles in fp8 (4 DoubleRow pairs)
NP8 = KO8 // 2           # 4 pairs
NQKV = 3 * EMBED
TOKENS = 16384
N_CORES = 8
M_CORE = TOKENS // N_CORES  # 2048
NB = NQKV // P              # 96
MS = 512
XCH = 4                     # k-subtiles per fp16 x chunk -> 6 chunks of 2MB
WSCALE = 32.0
NJUNK = 60                  # HAM-prewarm matmuls

f32 = mybir.dt.float32
f16 = mybir.dt.float16
f8 = mybir.dt.float8e4
F16 = np.float16
F8 = ml_dtypes.float8_e4m3
DR = mybir.MatmulPerfMode.DoubleRow

_CACHE = {}
LAST_RESULTS = None


def _build():
    nc = bacc.Bacc(get_trn_type() or "TRN2", target_bir_lowering=False, debug=False)
    x16_d = nc.dram_tensor("x16", (P, KO16, M_CORE), f16, kind="ExternalInput")
    x8_d = nc.dram_tensor("x8", (P, KO8, M_CORE), f8, kind="ExternalInput")
    w16_d = nc.dram_tensor("w16", (P, NB, KO16, P), f16, kind="ExternalInput")
    w8_d = nc.dram_tensor("w8", (P, NB, KO8, P), f8, kind="ExternalInput")
    out_d = nc.dram_tensor("outt", (P, NB, M_CORE), f32, kind="ExternalOutput")

    NCH = KO16 // XCH  # 6
    nms = M_CORE // MS  # 4
    with tile.TileContext(nc) as tc:
        with tc.tile_pool(name="xpool", bufs=1) as xpool, \
             tc.tile_pool(name="wpool", bufs=8) as wpool, \
             tc.tile_pool(name="pspool", bufs=8, space="PSUM") as pspool, \
             tc.tile_pool(name="opool", bufs=6) as opool:
            # ---- PE prewarm: junk matmuls on memset tiles release the HAM
            # clock gate during the initial DMA wait. Emitted first so they
            # head the PE queue; they depend only on the DVE memsets.
            jw = wpool.tile([P, P], f16, tag="jw", bufs=1, name="jw")
            jx = wpool.tile([P, 64], f16, tag="jx", bufs=1, name="jx")
            nc.gpsimd.memset(jw[:], 0.0)
            nc.gpsimd.memset(jx[:], 0.0)
            psj = pspool.tile([P, MS], f32, tag="ps", name="psj")
            for _ in range(NJUNK):
                nc.tensor.matmul(psj[:, 0:64], jw[:], jx[:], start=True,
                                 stop=True)

            # ---- x tiles; chunks 0,2,4 + x8 on the SP ring, 1,3,5 on ACT.
            x_ch = [
                xpool.tile([P, XCH, M_CORE], f16, tag=f"x{c}", name=f"x_ch{c}")
                for c in range(NCH)
            ]
            x8t = xpool.tile([P, KO8, M_CORE], f8, tag="x8", name="x8t")

            # SP ring: chunk 0 per-ko (finest pieces while the ring ramps)
            xc = x_ch[0]
            nc.sync.dma_start(xc[:, 0:1, 0:MS], x16_d[:, 0:1, 0:MS])
            nc.sync.dma_start(xc[:, 0:1, MS:], x16_d[:, 0:1, MS:])
            for k in (1, 2, 3):
                nc.sync.dma_start(xc[:, k:k + 1, :], x16_d[:, k:k + 1, :])
            for c in (2, 4):
                nc.sync.dma_start(
                    x_ch[c][:, 0:2, :], x16_d[:, c * XCH:c * XCH + 2, :]
                )
                nc.sync.dma_start(
                    x_ch[c][:, 2:XCH, :],
                    x16_d[:, c * XCH + 2:(c + 1) * XCH, :],
                )
            nc.sync.dma_start(x8t[:], x8_d[:])

            # ---- ACT ring: W heads, odd x chunks, W tails, w8, steady W.
            wgroup = {}        # nb -> [wt, wt8]

            def w_tile(nb):
                wt = wpool.tile([P, KO16, P], f16, tag="w", name="w_t")
                wgroup[nb] = [wt, None]
                return wt

            wt0 = w_tile(0)
            wt1 = w_tile(1)
            nc.scalar.dma_start(wt0[:, 0:4], w16_d[:, 0, 0:4])
            nc.scalar.dma_start(wt1[:, 0:4], w16_d[:, 1, 0:4])
            nc.scalar.dma_start(x_ch[1][:, 0:2, :], x16_d[:, 4:6, :])
            nc.scalar.dma_start(wt0[:, 4:], w16_d[:, 0, 4:])
            nc.scalar.dma_start(x_ch[1][:, 2:XCH, :], x16_d[:, 6:8, :])
            nc.scalar.dma_start(wt1[:, 4:], w16_d[:, 1, 4:])
            for c in (3, 5):
                nc.scalar.dma_start(
                    x_ch[c][:, 0:2, :], x16_d[:, c * XCH:c * XCH + 2, :]
                )
                nc.scalar.dma_start(
                    x_ch[c][:, 2:XCH, :],
                    x16_d[:, c * XCH + 2:(c + 1) * XCH, :],
                )

            def emit_w8(nb):
                wt8 = wpool.tile([P, KO8, P], f8, tag="w8", name="w8_t")
                nc.scalar.dma_start(wt8[:], w8_d[:, nb])
                wgroup[nb][1] = wt8

            def emit_w_dmas(nb):
                wt = w_tile(nb)
                nc.scalar.dma_start(wt[:], w16_d[:, nb])
                emit_w8(nb)

            emit_w8(0)
            emit_w8(1)

            pss_of = {}

            def emit_pss(nb):
                pss_of[nb] = [
                    pspool.tile([P, MS], f32, tag="ps", name="ps")
                    for _ in range(nms)
                ]

            def emit_mm(nb, kind, idx, ms):
                wt, wt8 = wgroup[nb]
                pss = pss_of[nb]
                if kind == "f":
                    xc = x_ch[idx // XCH]
                    kk = idx % XCH
                    nc.tensor.matmul(
                        pss[ms][:],
                        wt[:, idx],
                        xc[:, kk, ms * MS:(ms + 1) * MS],
                        start=(idx == 0),
                        stop=False,
                    )
                else:
                    nc.tensor.matmul(
                        pss[ms][:],
                        wt8[:, 2 * idx:2 * idx + 2, :],
                        x8t[:, 2 * idx:2 * idx + 2,
                            ms * MS:(ms + 1) * MS],
                        start=False,
                        stop=(idx == NP8 - 1),
                        perf_mode=DR,
                    )

            def emit_drain(nb):
                for ms in range(nms):
                    o_sb = opool.tile([P, MS], f32, tag="o", name="o_sb")
                    nc.vector.tensor_copy(o_sb[:], pss_of[nb][ms][:])
                    nc.sync.dma_start(
                        out_d[:, nb, ms * MS:(ms + 1) * MS],
                        o_sb[:],
                    )

            STEPS = [("f", ko) for ko in range(KO16)] + \
                    [("q", t) for t in range(NP8)]

            def emit_nb_mms(nb, ms_outer):
                # last nb: ms-outer so the psum drains stagger and the
                # final drain tail is one group, not four
                if ms_outer:
                    order = [(s, ms) for ms in range(nms) for s in STEPS]
                else:
                    order = [(s, ms) for s in STEPS for ms in range(nms)]
                for (kind, idx), ms in order:
                    emit_mm(nb, kind, idx, ms)

            # Warmup: nb0/nb1 interleaved; chunk 0 at ko granularity
            # (finest DMA-to-compute pipelining while the rings ramp),
            # chunk granularity after.
            emit_pss(0)
            emit_pss(1)
            for ko in range(XCH):
                for nb in (0, 1):
                    for ms in range(nms):
                        emit_mm(nb, "f", ko, ms)
            for c in range(1, NCH):
                for nb in (0, 1):
                    for ko in range(c * XCH, (c + 1) * XCH):
                        for ms in range(nms):
                            emit_mm(nb, "f", ko, ms)
            for nb in (0, 1):
                for t in range(NP8):
                    for ms in range(nms):
                        emit_mm(nb, "q", t, ms)

            emit_w_dmas(2)
            emit_drain(0)
            emit_drain(1)

            for nb in range(2, NB):
                if nb + 1 < NB:
                    # next nb's W DMAs stream one nb ahead; wpool WAR
                    # backpressure (bufs=8) paces the far-future tiles
                    emit_w_dmas(nb + 1)
                emit_pss(nb)
                emit_nb_mms(nb, ms_outer=(nb == NB - 1))
                emit_drain(nb)

    nc.compile()
    return nc


def kernel(hidden_states, qkv_proj, position_ids=None, **_unused):
    global LAST_RESULTS
    x = np.ascontiguousarray(hidden_states, dtype=np.float32).reshape(TOKENS, EMBED)
    w = np.ascontiguousarray(qkv_proj, dtype=np.float32)

    if "nc" not in _CACHE:
        _CACHE["nc"] = _build()
    nc = _CACHE["nc"]

    # w_t: [P, KO, NQKV] with w_t[p, ko, n] = 32*W[n, ko*128+p]
    w_t = np.ascontiguousarray(
        (w * np.float32(WSCALE)).T.reshape(KO, P, NQKV).transpose(1, 0, 2)
    )
    # nb-major: [P, NB, KOx, 128] so each per-nb DMA is contiguous
    w16 = np.ascontiguousarray(
        w_t[:, :KO16].reshape(P, KO16, NB, P).transpose(0, 2, 1, 3)
    ).astype(F16)
    w8 = np.ascontiguousarray(
        np.clip(w_t[:, KO16:], -240, 240).reshape(P, KO8, NB, P)
        .transpose(0, 2, 1, 3)
    ).astype(F8)
    in_maps = []
    for i in range(N_CORES):
        xs = x[i * M_CORE:(i + 1) * M_CORE]
        x_t = np.ascontiguousarray(
            xs.T.reshape(KO, P, M_CORE).transpose(1, 0, 2)
        )
        x16 = x_t[:, :KO16].astype(F16)
        x8 = np.clip(x_t[:, KO16:], -240, 240).astype(F8)
        in_maps.append({"x16": x16, "x8": x8, "w16": w16, "w8": w8})

    res = run_bass_kernel_spmd(nc, in_maps, core_ids=list(range(N_CORES)))
    LAST_RESULTS = res

    inv = np.float32(1.0 / WSCALE)
    parts = [
        res.results[i]["outt"].transpose(2, 1, 0).reshape(M_CORE, NQKV) * inv
        for i in range(N_CORES)
    ]
    qkv = np.concatenate(parts, axis=0)
    query = np.ascontiguousarray(qkv[:, :EMBED]).reshape(TOKENS, 32, 128)
    key = np.ascontiguousarray(qkv[:, EMBED:2 * EMBED]).reshape(TOKENS, 32, 128)
    value = np.ascontiguousarray(qkv[:, 2 * EMBED:]).reshape(TOKENS, 32, 128)
    return (query, key, value)
